# revision 1
# baseline (speedup 1.0000x reference)
"""Grouped GEMM (MoE routing) Trainium2 kernel.

Strategy: tensor-parallel shard of the output N dim across 8 NeuronCores.
Every core sees all T=8192 tokens and a 512-wide slice of every expert's
weights, so per-core work is identical regardless of segment sizes and a
single SPMD program (with the segment boundaries baked in as compile-time
constants) runs on all 8 cores.

Per core:  out_t[n, t] = sum_k w_t[e(t), k, n] * a_t[k, t]
  - a_t   : a transposed to [K, T]  (shared by all cores)
  - w_t   : per-core weight slices [E_active, K, 512] (K-major)
  - out_t : [512, T]; host concatenates along N and transposes back.

Matmul mapping: stationary lhsT = w_t tile [k=128, n=128], moving rhs =
a_t tile [k=128, tok<=512], PSUM out [n=128, tok<=512], accumulated over
the 32 k-chunks.  dtype float32r -> full-rate fp32 when moving dim >= 256,
so segments are split into even token pieces of 256..512.
"""

import numpy as np

import concourse.bacc as bacc
import concourse.bass as bass
import concourse.mybir as mybir
import concourse.tile as tile
from concourse.bass_utils import run_bass_kernel_spmd

NC = 8          # NeuronCores
P = 128         # partitions
TB = 512        # max token block (moving free dim, one PSUM bank of fp32)
KOC = 8         # k-chunks per a-tile DMA batch

LAST_RESULT = {}


def _token_blocks(seg_starts, seg_ends):
    """Split each segment into even pieces of <=512 tokens (>=256 when the
    segment allows, keeping float32r at full rate)."""
    blocks = []  # (tstart, tlen, active_expert_idx)
    for widx, (s, t) in enumerate(zip(seg_starts, seg_ends)):
        ln = t - s
        npieces = max(1, -(-ln // TB))
        base, rem = divmod(ln, npieces)
        p = s
        for i in range(npieces):
            L = base + (1 if i < rem else 0)
            if L > 0:
                blocks.append((p, L, widx))
                p += L
    return blocks


def _build_program(T, K, NS, EA, blocks):
    f32 = mybir.dt.float32
    f32r = mybir.dt.float32r
    KO = K // P
    NB = NS // P
    koc_n = min(KOC, KO)

    nc = bacc.Bacc(None, target_bir_lowering=False)
    at = nc.declare_dram_parameter("at", [KO, P, T], f32r, isOutput=False)
    wt = nc.declare_dram_parameter("wt", [EA, KO, P, NS], f32r, isOutput=False)
    ot = nc.declare_dram_parameter("ot", [NB, P, T], f32, isOutput=True)

    with tile.TileContext(nc) as tc:
        with (
            tc.tile_pool(name="wpool", bufs=2) as wpool,
            tc.tile_pool(name="apool", bufs=2) as apool,
            tc.tile_pool(name="opool", bufs=2) as opool,
            tc.tile_pool(name="psum", bufs=8, space=bass.MemorySpace.PSUM) as psum_pool,
        ):
            cur_widx = -1
            w_tile = None
            for (ts, L, widx) in blocks:
                # f32r matmuls need an even moving size: widen odd blocks by
                # one token for compute, write back only the real L columns.
                Lc = L + (L % 2)
                tsc = ts if ts + Lc <= T else ts - 1
                off = ts - tsc
                if widx != cur_widx:
                    w_tile = wpool.tile([P, KO, NS], f32r, tag="w", name="w_tile")
                    # one 8MB DMA: src (ko, kp, n) -> dst (kp, ko, n)
                    nc.sync.dma_start(
                        out=w_tile[:, :, :],
                        in_=wt[widx].transpose([1, 0, 2]),
                    )
                    cur_widx = widx
                ptiles = [psum_pool.tile([P, Lc], f32, tag="ps", name=f"ps{nb}",
                                         padded_shape=[P, TB])
                          for nb in range(NB)]
                for koc in range(KO // koc_n):
                    a_tile = apool.tile([P, koc_n, Lc], f32r, tag="a", name="a_tile",
                                        padded_shape=[P, koc_n, TB])
                    nc.sync.dma_start(
                        out=a_tile[:, :, :],
                        in_=at[koc * koc_n:(koc + 1) * koc_n, :, tsc:tsc + Lc]
                        .transpose([1, 0, 2]),
                    )
                    for koi in range(koc_n):
                        ko = koc * koc_n + koi
                        for nb in range(NB):
                            nc.tensor.matmul(
                                ptiles[nb][:, :],
                                w_tile[:, ko, nb * P:(nb + 1) * P],
                                a_tile[:, koi, :],
                                start=(ko == 0),
                                stop=(ko == KO - 1),
                            )
                o_tile = opool.tile([P, NB, L], f32, tag="o", name="o_tile",
                                    padded_shape=[P, NB, TB])
                for nb in range(NB):
                    nc.vector.tensor_copy(o_tile[:, nb, :], ptiles[nb][:, off:off + L])
                nc.sync.dma_start(
                    out=ot[:, :, ts:ts + L].transpose([1, 0, 2]),
                    in_=o_tile[:, :, :],
                )
    nc.compile()
    return nc


def kernel(a, b, c, seg_indptr, weight_indices, batch_size, **_):
    T, K = a.shape
    E, N, K2 = b.shape
    assert K == K2
    NS = N // NC

    seg = np.asarray(seg_indptr).astype(np.int64)
    widx_arr = np.asarray(weight_indices).astype(np.int64)
    segs = [(int(seg[e]), int(seg[e + 1]), int(widx_arr[e]))
            for e in range(int(batch_size)) if seg[e + 1] > seg[e]]
    seg_starts = [s for s, _, _ in segs]
    seg_ends = [t for _, t, _ in segs]
    experts = [w for _, _, w in segs]
    EA = len(segs)
    blocks = _token_blocks(seg_starts, seg_ends)

    a = np.ascontiguousarray(a, dtype=np.float32)
    at_np = np.ascontiguousarray(a.T).reshape(K // P, P, T)

    KO = K // P
    in_maps = []
    for j in range(NC):
        w = np.empty((EA, KO, P, NS), dtype=np.float32)
        for ei, e in enumerate(experts):
            # b[e] is [N, K] row-major; out = a @ b[e].T needs W^T = [K, NS]
            w[ei] = np.ascontiguousarray(
                b[e][j * NS:(j + 1) * NS, :].T
            ).reshape(KO, P, NS)
        in_maps.append({"at": at_np, "wt": w})

    nc = _build_program(T, K, NS, EA, blocks)

    import os
    trace = bool(int(os.environ.get("BASS_KERNEL_TRACE", "0")))
    res = run_bass_kernel_spmd(nc, in_maps, list(range(NC)), trace=trace)
    LAST_RESULT["exec_time_ns"] = res.exec_time_ns
    LAST_RESULT["results"] = res

    out_t = np.empty((N, T), dtype=np.float32)
    for j in range(NC):
        out_t[j * NS:(j + 1) * NS] = res.results[j]["ot"].reshape(NS, T)
    return np.ascontiguousarray(out_t.T)



# revision 2
# speedup vs baseline: 1.5432x; 1.5432x over previous
"""Grouped GEMM (MoE routing) Trainium2 kernel.

Strategy: tensor-parallel shard of the output N dim across 8 NeuronCores.
Every core sees all T=8192 tokens and a 512-wide slice of every expert's
weights, so per-core work is identical regardless of segment sizes and a
single SPMD program (with the segment boundaries baked in as compile-time
constants) runs on all 8 cores.

Per core:  out_t[n, t] = sum_k w_t[e(t), k, n] * a_t[k, t]

All operands are bf16 (well within the 2e-2 rel-err budget): this halves
HBM traffic vs fp32 (117 MB/core) and keeps the matmul at 1 cycle/row,
making the kernel compute-bound at ~437us/core (1.05M PE rows @ 2.4GHz).

DMA efficiency: every DRAM region is pre-tiled on the host into the exact
SBUF consumption order, so each dma_start is one fully-contiguous 1-4 MB
transfer (split by HW across all 16 SDMA engines at ~340-420 GB/s),
instead of the 2KB/row strided pattern a [K,T] layout would produce.

Blocks of up to 1024 tokens are processed as two <=512 halves sharing
each stationary weight tile (halving LoadStationary count); the 4 n-chunks
x 2 halves occupy all 8 PSUM banks.
"""

import numpy as np
from ml_dtypes import bfloat16

import concourse.bacc as bacc
import concourse.bass as bass
import concourse.mybir as mybir
import concourse.tile as tile
from concourse.bass_utils import run_bass_kernel_spmd

NC = 8          # NeuronCores
P = 128         # partitions
TB = 1024       # max token block (two <=512 halves -> 8 live PSUM banks)
KOC = 8         # k-chunks per a-tile DMA batch

LAST_RESULT = {}


def _segments(seg_indptr, weight_indices, batch_size, T):
    """Token segments per reference semantics: token t uses expert slot
    clip(searchsorted(indptr, t, 'right')-1, 0, bs-1)."""
    seg = np.asarray(seg_indptr).astype(np.int64)
    widx = np.asarray(weight_indices).astype(np.int64)
    bs = int(batch_size)
    segs = []
    for e in range(bs):
        s = 0 if e == 0 else int(min(max(seg[e], 0), T))
        t = T if e == bs - 1 else int(min(max(seg[e + 1], 0), T))
        if t > s:
            segs.append((s, t, int(widx[e])))
    return segs


def _token_blocks(segs):
    """Split each segment into even-count pieces of <=TB tokens."""
    blocks = []  # (tstart, tlen, run_idx) with run_idx = weight-load slot
    run = -1
    for (s, t, _w) in segs:
        run += 1
        ln = t - s
        npieces = max(1, -(-ln // TB))
        base, rem = divmod(ln, npieces)
        p = s
        for i in range(npieces):
            L = base + (1 if i < rem else 0)
            if L > 0:
                blocks.append((p, L, run))
                p += L
    return blocks


def _build_program(T, K, NS, EA, blocks):
    f32 = mybir.dt.float32
    bf16 = mybir.dt.bfloat16
    KO = K // P
    NB = NS // P
    koc_n = min(KOC, KO)

    nc = bacc.Bacc(None, target_bir_lowering=False)
    at = nc.declare_dram_parameter("at", [P, KO * T], bf16, isOutput=False)
    wt = nc.declare_dram_parameter("wt", [EA, P, KO * NS], bf16, isOutput=False)
    ot = nc.declare_dram_parameter("ot", [P, NB * T], f32, isOutput=True)

    with tile.TileContext(nc) as tc:
        with (
            tc.tile_pool(name="wpool", bufs=2) as wpool,
            tc.tile_pool(name="apool", bufs=2) as apool,
            tc.tile_pool(name="opool", bufs=2) as opool,
            tc.tile_pool(name="psum", bufs=8, space=bass.MemorySpace.PSUM) as psum_pool,
        ):
            cur_run = -1
            w_tile = None
            for (ts, L, run) in blocks:
                if run != cur_run:
                    w_tile = wpool.tile([P, KO * NS], bf16, tag="w", name="w_tile")
                    # one fully-contiguous 4MB DMA
                    nc.sync.dma_start(out=w_tile[:, :], in_=wt[run])
                    cur_run = run
                if L > P * 4:  # split into two halves sharing stationaries
                    h0 = (L + 1) // 2
                    h0 += h0 % 2
                    halves = [(0, h0), (h0, L - h0)]
                else:
                    halves = [(0, L)]
                ptiles = [
                    [psum_pool.tile([P, hl], f32, tag="ps", name=f"ps{nb}h{h}",
                                    padded_shape=[P, 512])
                     for h, (hs, hl) in enumerate(halves)]
                    for nb in range(NB)
                ]
                for cb in range(KO // koc_n):
                    a_tile = apool.tile([P, koc_n * L], bf16, tag="a",
                                        name="a_tile",
                                        padded_shape=[P, koc_n * TB])
                    base = KO * ts + cb * koc_n * L
                    nc.sync.dma_start(
                        out=a_tile[:, :],
                        in_=at[:, base:base + koc_n * L],
                    )
                    for koi in range(koc_n):
                        ko = cb * koc_n + koi
                        for nb in range(NB):
                            stat = w_tile[:, ko * NS + nb * P:ko * NS + (nb + 1) * P]
                            for h, (hs, hl) in enumerate(halves):
                                nc.tensor.matmul(
                                    ptiles[nb][h][:, :],
                                    stat,
                                    a_tile[:, koi * L + hs:koi * L + hs + hl],
                                    start=(ko == 0),
                                    stop=(ko == KO - 1),
                                )
                o_tile = opool.tile([P, NB * L], f32, tag="o", name="o_tile",
                                    padded_shape=[P, NB * TB])
                for nb in range(NB):
                    for h, (hs, hl) in enumerate(halves):
                        nc.vector.tensor_copy(
                            o_tile[:, nb * L + hs:nb * L + hs + hl],
                            ptiles[nb][h][:, :hl],
                        )
                nc.sync.dma_start(
                    out=ot[:, NB * ts:NB * (ts + L)],
                    in_=o_tile[:, :],
                )
    nc.compile()
    return nc


def kernel(a, b, c, seg_indptr, weight_indices, batch_size, **_):
    T, K = a.shape
    E, N, K2 = b.shape
    assert K == K2
    NS = N // NC
    KO = K // P
    NB = NS // P
    koc_n = min(KOC, KO)

    segs = _segments(seg_indptr, weight_indices, batch_size, T)
    blocks = _token_blocks(segs)
    run_experts = [w for (_, _, w) in segs]  # expert id per weight-load run
    EA = len(run_experts)

    # a: [T, K] fp32 -> bf16, pre-tiled to [P, KO*T] in consumption order
    abf = a.astype(bfloat16)
    at_kpt = np.ascontiguousarray(abf.T).reshape(KO, P, T)  # [ko, p, t]
    at_flat = np.empty((P, KO * T), dtype=bfloat16)
    for (ts, L, _) in blocks:
        for cb in range(KO // koc_n):
            slab = at_kpt[cb * koc_n:(cb + 1) * koc_n, :, ts:ts + L]
            at_flat[:, KO * ts + cb * koc_n * L:KO * ts + (cb + 1) * koc_n * L] = \
                slab.transpose(1, 0, 2).reshape(P, koc_n * L)

    # b: [E, N, K] fp32 -> bf16 [E, NC, P(k%128), KO, NS(n)]
    bbf = b.astype(bfloat16)
    wt_all = np.ascontiguousarray(
        bbf.reshape(E, NC, NS, KO, P).transpose(0, 1, 4, 3, 2)
    ).reshape(E, NC, P, KO * NS)

    in_maps = []
    for j in range(NC):
        wtj = np.ascontiguousarray(wt_all[run_experts, j])  # [EA, P, KO*NS]
        in_maps.append({"at": at_flat, "wt": wtj})

    nc = _build_program(T, K, NS, EA, blocks)

    import os
    trace = bool(int(os.environ.get("BASS_KERNEL_TRACE", "0")))
    res = run_bass_kernel_spmd(nc, in_maps, list(range(NC)), trace=trace)
    LAST_RESULT["exec_time_ns"] = res.exec_time_ns
    LAST_RESULT["results"] = res

    out = np.empty((T, N), dtype=np.float32)
    for j in range(NC):
        otj = res.results[j]["ot"]  # [P, NB*T] fp32
        for (ts, L, _) in blocks:
            arr = otj[:, NB * ts:NB * (ts + L)].reshape(P, NB, L)
            out[ts:ts + L, j * NS:(j + 1) * NS] = \
                arr.transpose(2, 1, 0).reshape(L, NS)
    return out


# revision 5
# speedup vs baseline: 1.6780x; 1.0873x over previous
"""Grouped GEMM (MoE routing) Trainium2 kernel.

Strategy: tensor-parallel shard of the output N dim across 8 NeuronCores.
Every core sees all T=8192 tokens and a 512-wide slice of every expert's
weights, so per-core work is identical regardless of segment sizes and a
single SPMD program (with the segment boundaries baked in as compile-time
constants) runs on all 8 cores.

Per core:  out_t[n, t] = sum_k w_t[e(t), k, n] * a_t[k, t]

All operands are bf16 (well within the 2e-2 rel-err budget): this halves
HBM traffic vs fp32 (117 MB/core) and keeps the matmul at 1 cycle/row,
making the kernel compute-bound at ~437us/core (1.05M PE rows @ 2.4GHz).

DMA efficiency: every DRAM region is pre-tiled on the host into the exact
SBUF consumption order, so each dma_start is one fully-contiguous 1-4 MB
transfer (split by HW across all 16 SDMA engines at ~340-420 GB/s),
instead of the 2KB/row strided pattern a [K,T] layout would produce.

Blocks of up to 1024 tokens are processed as two <=512 halves sharing
each stationary weight tile (halving LoadStationary count); the 4 n-chunks
x 2 halves occupy all 8 PSUM banks.
"""

import numpy as np
from ml_dtypes import bfloat16

import concourse.bacc as bacc
import concourse.bass as bass
import concourse.mybir as mybir
import concourse.tile as tile
from concourse.bass_utils import run_bass_kernel_spmd

NC = 8          # NeuronCores
P = 128         # partitions
TB = 1024       # max token block (two <=512 halves -> 8 live PSUM banks)
KOC = 8         # k-chunks per a-tile DMA batch

LAST_RESULT = {}


def _segments(seg_indptr, weight_indices, batch_size, T):
    """Token segments per reference semantics: token t uses expert slot
    clip(searchsorted(indptr, t, 'right')-1, 0, bs-1)."""
    seg = np.asarray(seg_indptr).astype(np.int64)
    widx = np.asarray(weight_indices).astype(np.int64)
    bs = int(batch_size)
    segs = []
    for e in range(bs):
        s = 0 if e == 0 else int(min(max(seg[e], 0), T))
        t = T if e == bs - 1 else int(min(max(seg[e + 1], 0), T))
        if t > s:
            segs.append((s, t, int(widx[e])))
    return segs


def _token_blocks(segs):
    """Split each segment into even-count pieces of <=TB tokens."""
    blocks = []  # (tstart, tlen, run_idx) with run_idx = weight-load slot
    run = -1
    for (s, t, _w) in segs:
        run += 1
        ln = t - s
        npieces = max(1, -(-ln // TB))
        base, rem = divmod(ln, npieces)
        p = s
        for i in range(npieces):
            L = base + (1 if i < rem else 0)
            if L > 0:
                blocks.append((p, L, run))
                p += L
    return blocks


def _build_program(T, K, NS, EA, blocks):
    f32 = mybir.dt.float32
    bf16 = mybir.dt.bfloat16
    KO = K // P
    NB = NS // P
    koc_n = min(KOC, KO)

    nc = bacc.Bacc(None, target_bir_lowering=False)
    at = nc.declare_dram_parameter("at", [P, KO * T], bf16, isOutput=False)
    wt = nc.declare_dram_parameter("wt", [EA, P, KO * NS], bf16, isOutput=False)
    ot = nc.declare_dram_parameter("ot", [P, NB * T], f32, isOutput=True)

    with tile.TileContext(nc) as tc:
        with (
            tc.tile_pool(name="wpool", bufs=3) as wpool,
            tc.tile_pool(name="apool", bufs=3) as apool,
            tc.tile_pool(name="opool", bufs=2) as opool,
            tc.tile_pool(name="psum", bufs=8, space=bass.MemorySpace.PSUM) as psum_pool,
        ):
            cur_run = -1
            w_tile = None
            for (ts, L, run) in blocks:
                if run != cur_run:
                    w_tile = wpool.tile([P, KO * NS], bf16, tag="w", name="w_tile")
                    # one fully-contiguous 4MB DMA; scalar (ACT) HWDGE ring so
                    # weight prefetch never queues behind the a-tile FIFO
                    nc.scalar.dma_start(out=w_tile[:, :], in_=wt[run])
                    cur_run = run
                if L > P * 4:  # split into two halves sharing stationaries
                    h0 = (L + 1) // 2
                    h0 += h0 % 2
                    halves = [(0, h0), (h0, L - h0)]
                else:
                    halves = [(0, L)]
                ptiles = [
                    [psum_pool.tile([P, hl], f32, tag="ps", name=f"ps{nb}h{h}",
                                    padded_shape=[P, 512])
                     for h, (hs, hl) in enumerate(halves)]
                    for nb in range(NB)
                ]
                for cb in range(KO // koc_n):
                    a_tile = apool.tile([P, koc_n * L], bf16, tag="a",
                                        name="a_tile",
                                        padded_shape=[P, koc_n * TB])
                    base = KO * ts + cb * koc_n * L
                    nc.sync.dma_start(
                        out=a_tile[:, :],
                        in_=at[:, base:base + koc_n * L],
                    )
                    for koi in range(koc_n):
                        ko = cb * koc_n + koi
                        for nb in range(NB):
                            stat = w_tile[:, ko * NS + nb * P:ko * NS + (nb + 1) * P]
                            for h, (hs, hl) in enumerate(halves):
                                nc.tensor.matmul(
                                    ptiles[nb][h][:, :],
                                    stat,
                                    a_tile[:, koi * L + hs:koi * L + hs + hl],
                                    start=(ko == 0),
                                    stop=(ko == KO - 1),
                                )
                o_tile = opool.tile([P, NB * L], f32, tag="o", name="o_tile",
                                    padded_shape=[P, NB * TB])
                for nb in range(NB):
                    for h, (hs, hl) in enumerate(halves):
                        nc.vector.tensor_copy(
                            o_tile[:, nb * L + hs:nb * L + hs + hl],
                            ptiles[nb][h][:, :hl],
                        )
                # SWDGE so output stores don't share a FIFO with loads
                nc.gpsimd.dma_start(
                    out=ot[:, NB * ts:NB * (ts + L)],
                    in_=o_tile[:, :],
                )
    nc.compile()
    return nc


def kernel(a, b, c, seg_indptr, weight_indices, batch_size, **_):
    T, K = a.shape
    E, N, K2 = b.shape
    assert K == K2
    NS = N // NC
    KO = K // P
    NB = NS // P
    koc_n = min(KOC, KO)

    segs = _segments(seg_indptr, weight_indices, batch_size, T)
    blocks = _token_blocks(segs)
    run_experts = [w for (_, _, w) in segs]  # expert id per weight-load run
    EA = len(run_experts)

    # a: [T, K] fp32 -> bf16, pre-tiled to [P, KO*T] in consumption order
    abf = a.astype(bfloat16)
    at_kpt = np.ascontiguousarray(abf.T).reshape(KO, P, T)  # [ko, p, t]
    at_flat = np.empty((P, KO * T), dtype=bfloat16)
    for (ts, L, _) in blocks:
        for cb in range(KO // koc_n):
            slab = at_kpt[cb * koc_n:(cb + 1) * koc_n, :, ts:ts + L]
            at_flat[:, KO * ts + cb * koc_n * L:KO * ts + (cb + 1) * koc_n * L] = \
                slab.transpose(1, 0, 2).reshape(P, koc_n * L)

    # b: [E, N, K] fp32 -> bf16 [E, NC, P(k%128), KO, NS(n)]
    bbf = b.astype(bfloat16)
    wt_all = np.ascontiguousarray(
        bbf.reshape(E, NC, NS, KO, P).transpose(0, 1, 4, 3, 2)
    ).reshape(E, NC, P, KO * NS)

    in_maps = []
    for j in range(NC):
        wtj = np.ascontiguousarray(wt_all[run_experts, j])  # [EA, P, KO*NS]
        in_maps.append({"at": at_flat, "wt": wtj})

    nc = _build_program(T, K, NS, EA, blocks)

    import os
    trace = bool(int(os.environ.get("BASS_KERNEL_TRACE", "0")))
    res = run_bass_kernel_spmd(nc, in_maps, list(range(NC)), trace=trace)
    LAST_RESULT["exec_time_ns"] = res.exec_time_ns
    LAST_RESULT["results"] = res

    out = np.empty((T, N), dtype=np.float32)
    for j in range(NC):
        otj = res.results[j]["ot"]  # [P, NB*T] fp32
        for (ts, L, _) in blocks:
            arr = otj[:, NB * ts:NB * (ts + L)].reshape(P, NB, L)
            out[ts:ts + L, j * NS:(j + 1) * NS] = \
                arr.transpose(2, 1, 0).reshape(L, NS)
    return out


# revision 6
# speedup vs baseline: 1.6942x; 1.0097x over previous
"""Grouped GEMM (MoE routing) Trainium2 kernel.

Strategy: tensor-parallel shard of the output N dim across 8 NeuronCores.
Every core sees all T=8192 tokens and a 512-wide slice of every expert's
weights, so per-core work is identical regardless of segment sizes and a
single SPMD program (with the segment boundaries baked in as compile-time
constants) runs on all 8 cores.

Per core:  out_t[n, t] = sum_k w_t[e(t), k, n] * a_t[k, t]

All operands are bf16 (well within the 2e-2 rel-err budget): this halves
HBM traffic vs fp32 (~109 MB/core) and keeps the matmul at 1 cycle/row,
making the kernel compute-bound at ~437us/core (1.05M PE rows @ 2.4GHz).

DMA plan: every DRAM region is pre-tiled on the host into the exact SBUF
consumption order, so each dma_start is one fully-contiguous 0.25-2 MB
transfer (split by HW across all 16 SDMA engines). Three separate DMA
paths so nothing queues behind anything else: a-tiles on the sync HWDGE
ring, weight chunks on the scalar (ACT) HWDGE ring, output stores on
gpsimd (SWDGE). Weights load as 4 x 1MB chunks in a 12-buffer ring, so
prefetch is paced by compute progress ~3 segments ahead and the per-
switch HBM spike is 1MB, not 4MB.

Blocks of up to 1024 tokens are processed as two <=512 halves sharing
each stationary weight tile (halving LoadStationary count); the 4 n-chunks
x 2 halves occupy all 8 PSUM banks. A small lead block lets the first
matmul start ~8us in; a small tail block keeps the final copy+store off
the critical path.
"""

import numpy as np
from ml_dtypes import bfloat16

import concourse.bacc as bacc
import concourse.bass as bass
import concourse.mybir as mybir
import concourse.tile as tile
from concourse.bass_utils import run_bass_kernel_spmd

NC = 8          # NeuronCores
P = 128         # partitions
TB = 1024       # max token block (two <=512 halves -> 8 live PSUM banks)
KOC = 8         # k-chunks per a-tile / w-chunk DMA batch
LEAD = 256      # lead/tail block size

LAST_RESULT = {}


def _segments(seg_indptr, weight_indices, batch_size, T):
    """Token segments per reference semantics: token t uses expert slot
    clip(searchsorted(indptr, t, 'right')-1, 0, bs-1)."""
    seg = np.asarray(seg_indptr).astype(np.int64)
    widx = np.asarray(weight_indices).astype(np.int64)
    bs = int(batch_size)
    segs = []
    for e in range(bs):
        s = 0 if e == 0 else int(min(max(seg[e], 0), T))
        t = T if e == bs - 1 else int(min(max(seg[e + 1], 0), T))
        if t > s:
            segs.append((s, t, int(widx[e])))
    return segs


def _token_blocks(segs):
    """Split each segment into even-count pieces of <=TB tokens; carve a
    small lead (first) and tail (last) block for pipeline ramp/drain."""
    blocks = []  # (tstart, tlen, run_idx) with run_idx = weight-load slot
    run = -1
    for (s, t, _w) in segs:
        run += 1
        ln = t - s
        npieces = max(1, -(-ln // TB))
        base, rem = divmod(ln, npieces)
        p = s
        for i in range(npieces):
            L = base + (1 if i < rem else 0)
            if L > 0:
                blocks.append((p, L, run))
                p += L
    if blocks and blocks[0][1] >= 2 * LEAD:
        ts, L, r = blocks[0]
        blocks[0:1] = [(ts, LEAD, r), (ts + LEAD, L - LEAD, r)]
    if blocks and blocks[-1][1] >= 2 * LEAD:
        ts, L, r = blocks[-1]
        blocks[-1:] = [(ts, L - LEAD, r), (ts + L - LEAD, LEAD, r)]
    return blocks


def _build_program(T, K, NS, EA, blocks):
    f32 = mybir.dt.float32
    bf16 = mybir.dt.bfloat16
    KO = K // P
    NB = NS // P
    koc_n = min(KOC, KO)
    NCB = KO // koc_n          # w chunks / a-tile batches per block
    WCH = koc_n * NS           # w chunk elems per partition

    nc = bacc.Bacc(None, target_bir_lowering=False)
    at = nc.declare_dram_parameter("at", [P, KO * T], bf16, isOutput=False)
    wt = nc.declare_dram_parameter("wt", [EA, P, KO * NS], bf16, isOutput=False)
    ot = nc.declare_dram_parameter("ot", [P, NB * T], bf16, isOutput=True)

    with tile.TileContext(nc) as tc:
        with (
            tc.tile_pool(name="wpool", bufs=3 * NCB) as wpool,
            tc.tile_pool(name="apool", bufs=3) as apool,
            tc.tile_pool(name="opool", bufs=2) as opool,
            tc.tile_pool(name="psum", bufs=8, space=bass.MemorySpace.PSUM) as psum_pool,
        ):
            cur_run = -1
            wchunks = None
            for (ts, L, run) in blocks:
                if run != cur_run:
                    wchunks = []
                    for cb in range(NCB):
                        wc = wpool.tile([P, WCH], bf16, tag="w",
                                        name=f"w{cb}")
                        # contiguous 1MB on the scalar (ACT) HWDGE ring so
                        # weight prefetch never queues behind a-tiles
                        nc.scalar.dma_start(
                            out=wc[:, :],
                            in_=wt[run][:, cb * WCH:(cb + 1) * WCH],
                        )
                        wchunks.append(wc)
                    cur_run = run
                if L > 512:  # two halves sharing stationaries
                    h0 = (L + 1) // 2
                    h0 += h0 % 2
                    halves = [(0, h0), (h0, L - h0)]
                else:
                    halves = [(0, L)]
                ptiles = [
                    [psum_pool.tile([P, hl], f32, tag="ps", name=f"ps{nb}h{h}",
                                    padded_shape=[P, 512])
                     for h, (hs, hl) in enumerate(halves)]
                    for nb in range(NB)
                ]
                for cb in range(NCB):
                    a_tile = apool.tile([P, koc_n * L], bf16, tag="a",
                                        name="a_tile",
                                        padded_shape=[P, koc_n * TB])
                    base = KO * ts + cb * koc_n * L
                    nc.sync.dma_start(
                        out=a_tile[:, :],
                        in_=at[:, base:base + koc_n * L],
                    )
                    for koi in range(koc_n):
                        ko = cb * koc_n + koi
                        for nb in range(NB):
                            stat = wchunks[cb][:, koi * NS + nb * P:
                                               koi * NS + (nb + 1) * P]
                            for h, (hs, hl) in enumerate(halves):
                                nc.tensor.matmul(
                                    ptiles[nb][h][:, :],
                                    stat,
                                    a_tile[:, koi * L + hs:koi * L + hs + hl],
                                    start=(ko == 0),
                                    stop=(ko == KO - 1),
                                )
                o_tile = opool.tile([P, NB * L], bf16, tag="o", name="o_tile",
                                    padded_shape=[P, NB * TB])
                for nb in range(NB):
                    for h, (hs, hl) in enumerate(halves):
                        nc.vector.tensor_copy(
                            o_tile[:, nb * L + hs:nb * L + hs + hl],
                            ptiles[nb][h][:, :hl],
                        )
                # SWDGE so output stores don't share a FIFO with loads
                nc.gpsimd.dma_start(
                    out=ot[:, NB * ts:NB * (ts + L)],
                    in_=o_tile[:, :],
                )
    nc.compile()
    return nc


def kernel(a, b, c, seg_indptr, weight_indices, batch_size, **_):
    T, K = a.shape
    E, N, K2 = b.shape
    assert K == K2
    NS = N // NC
    KO = K // P
    NB = NS // P
    koc_n = min(KOC, KO)

    segs = _segments(seg_indptr, weight_indices, batch_size, T)
    blocks = _token_blocks(segs)
    run_experts = [w for (_, _, w) in segs]  # expert id per weight-load run
    EA = len(run_experts)

    # a: [T, K] fp32 -> bf16, pre-tiled to [P, KO*T] in consumption order
    abf = a.astype(bfloat16)
    at_kpt = np.ascontiguousarray(abf.T).reshape(KO, P, T)  # [ko, p, t]
    at_flat = np.empty((P, KO * T), dtype=bfloat16)
    for (ts, L, _) in blocks:
        for cb in range(KO // koc_n):
            slab = at_kpt[cb * koc_n:(cb + 1) * koc_n, :, ts:ts + L]
            at_flat[:, KO * ts + cb * koc_n * L:KO * ts + (cb + 1) * koc_n * L] = \
                slab.transpose(1, 0, 2).reshape(P, koc_n * L)

    # b: [E, N, K] fp32 -> bf16 [E, NC, P(k%128), KO, NS(n)]
    bbf = b.astype(bfloat16)
    wt_all = np.ascontiguousarray(
        bbf.reshape(E, NC, NS, KO, P).transpose(0, 1, 4, 3, 2)
    ).reshape(E, NC, P, KO * NS)

    in_maps = []
    for j in range(NC):
        wtj = np.ascontiguousarray(wt_all[run_experts, j])  # [EA, P, KO*NS]
        in_maps.append({"at": at_flat, "wt": wtj})

    nc = _build_program(T, K, NS, EA, blocks)

    import os
    trace = bool(int(os.environ.get("BASS_KERNEL_TRACE", "0")))
    res = run_bass_kernel_spmd(nc, in_maps, list(range(NC)), trace=trace)
    LAST_RESULT["exec_time_ns"] = res.exec_time_ns
    LAST_RESULT["results"] = res

    out = np.empty((T, N), dtype=np.float32)
    for j in range(NC):
        otj = res.results[j]["ot"]  # [P, NB*T] bf16
        for (ts, L, _) in blocks:
            arr = otj[:, NB * ts:NB * (ts + L)].reshape(P, NB, L)
            out[ts:ts + L, j * NS:(j + 1) * NS] = \
                arr.transpose(2, 1, 0).reshape(L, NS).astype(np.float32)
    return out
